# revision 1
# baseline (speedup 1.0000x reference)
"""LIF neuron scan kernel for Trainium2 (Bass/Tile), SPMD over 8 NeuronCores.

Reference computation (T=32, B=16, N=65536, f32):
    m = 0
    for t in range(T):
        m = 0.25 * m + x[t]          # membrane update (beta = 0.25)
        spike[t] = (m >= 1.0)        # heaviside
        membrane[t] = m              # recorded pre-reset
        m = m - spike[t]             # soft reset (threshold = 1.0)
    return spikes, membranes

Sharding: split N across the 8 cores (N/8 = 8192 per core). The scan
recurrence is over T only, so each core runs an independent sequential
scan over its (T, B, 8192) slice with zero communication.

Per-core layout: the (B=16, 8192) plane per timestep flattens to
(128, 1024) — partition dim 128, 1024 contiguous f32 per partition
(4 KiB DMA lines). Timesteps are processed in blocks of TB=4; block
transfers are split so each DMA is 0.5-1 MiB (past the DMA setup knee
while keeping the pipeline fine-grained).

Spikes travel as uint8 on the wire (0/1 exact) and are converted back
to f32 on the host, cutting output bytes by 3/8. Membranes stay f32 —
the scan is bit-exact vs the f32 reference.

All DMA is HWDGE: input loads on the SP ring, output stores on the ACT
ring, so loads are never queued behind stores.
"""

import os

import numpy as np

import concourse.bacc as bacc
import concourse.mybir as mybir
import concourse.tile as tile
from concourse.bass_utils import run_bass_kernel_spmd
from concourse.mybir import AluOpType

BETA = 0.25
THRESHOLD = 1.0

T, B, N = 32, 16, 65536
NCORES = 8
NS = N // NCORES          # 8192 columns per core
P = 128                   # SBUF partitions
F = (B * NS) // P         # 1024 free-dim elements per partition
TB = 4                    # timesteps per SBUF block
SPLIT_X = 4               # x-load pieces per block   (4 -> 512 KiB per DMA)
SPLIT_MEM = 4             # membrane-store pieces     (4 -> 512 KiB per DMA)

_cache = {}


def _build_nc():
    nc = bacc.Bacc("TRN2", target_bir_lowering=False, debug=False)
    f32 = mybir.dt.float32
    u8 = mybir.dt.uint8
    x_d = nc.dram_tensor("x", [T, P, F], f32, kind="ExternalInput").ap()
    spk_d = nc.dram_tensor("spikes", [T, P, F], u8, kind="ExternalOutput").ap()
    mem_d = nc.dram_tensor("membranes", [T, P, F], f32, kind="ExternalOutput").ap()

    with tile.TileContext(nc) as tc:
        with (
            tc.tile_pool(name="xin", bufs=4) as xp,
            tc.tile_pool(name="mstate", bufs=1) as mp,
            tc.tile_pool(name="mem", bufs=3) as memp,
            tc.tile_pool(name="spk", bufs=3) as spkp,
        ):
            m = mp.tile([P, F], f32)
            for blk in range(T // TB):
                t0 = blk * TB
                xt = xp.tile([P, TB * F], f32)
                xstep = TB // SPLIT_X
                for s in range(0, TB, xstep):
                    nc.sync.dma_start(
                        xt[:, s * F : (s + xstep) * F].rearrange(
                            "p (t f) -> p t f", t=xstep
                        ),
                        x_d[t0 + s : t0 + s + xstep].rearrange("t p f -> p t f"),
                    )
                mem = memp.tile([P, TB * F], f32)
                spk = spkp.tile([P, TB * F], u8)
                for i in range(TB):
                    t = t0 + i
                    sl = slice(i * F, (i + 1) * F)
                    if t == 0:
                        # m starts at 0: first pre-reset membrane is x[0].
                        mpre = xt[:, sl]
                        nc.scalar.copy(mem[:, sl], xt[:, sl])
                    else:
                        mpre = mem[:, sl]
                        nc.vector.scalar_tensor_tensor(
                            mpre, m[:], BETA, xt[:, sl],
                            AluOpType.mult, AluOpType.add,
                        )
                    nc.vector.tensor_scalar(
                        spk[:, sl], mpre, THRESHOLD, None, AluOpType.is_ge
                    )
                    nc.vector.tensor_tensor(
                        m[:], mpre, spk[:, sl], AluOpType.subtract
                    )
                mstep = TB // SPLIT_MEM
                for s in range(0, TB, mstep):
                    nc.scalar.dma_start(
                        mem_d[t0 + s : t0 + s + mstep].rearrange("t p f -> p t f"),
                        mem[:, s * F : (s + mstep) * F].rearrange(
                            "p (t f) -> p t f", t=mstep
                        ),
                    )
                nc.scalar.dma_start(
                    spk_d[t0 : t0 + TB].rearrange("t p f -> p t f"),
                    spk[:].rearrange("p (t f) -> p t f", t=TB),
                )
    nc.finalize()  # run Bacc passes (reg alloc, sync-wait splitting)
    return nc


last_results = None  # BassKernelResults of the most recent run (for profiling)


def kernel(x: np.ndarray):
    global last_results
    assert x.shape == (T, B, N) and x.dtype == np.float32

    if "nc" not in _cache:
        _cache["nc"] = _build_nc()
    nc = _cache["nc"]

    in_maps = [
        {"x": np.ascontiguousarray(x[:, :, c * NS : (c + 1) * NS]).reshape(T, P, F)}
        for c in range(NCORES)
    ]
    trace = bool(int(os.environ.get("LIF_TRACE", "0")))
    if not trace:
        # NTFF tracing needs antenv.axon_hooks, which this container does
        # not ship — make sure a stray BASS_TRACE=1 can't crash the run.
        os.environ["BASS_NEVER_TRACE"] = "1"
    res = run_bass_kernel_spmd(
        nc,
        in_maps,
        core_ids=list(range(NCORES)),
        trace=trace,
    )
    last_results = res

    spikes = np.empty((T, B, N), dtype=np.float32)
    membranes = np.empty((T, B, N), dtype=np.float32)
    for c in range(NCORES):
        spikes[:, :, c * NS : (c + 1) * NS] = (
            res.results[c]["spikes"].astype(np.float32).reshape(T, B, NS)
        )
        membranes[:, :, c * NS : (c + 1) * NS] = res.results[c]["membranes"].reshape(
            T, B, NS
        )
    return spikes, membranes



# revision 9
# speedup vs baseline: 1.3242x; 1.3242x over previous
"""LIF neuron scan kernel for Trainium2 (Bass/Tile), SPMD over 8 NeuronCores.

Reference computation (T=32, B=16, N=65536, f32):
    m = 0
    for t in range(T):
        m = 0.25 * m + x[t]          # membrane update (beta = 0.25)
        spike[t] = (m >= 1.0)        # heaviside
        membrane[t] = m              # recorded pre-reset
        m = m - spike[t]             # soft reset (threshold = 1.0)
    return spikes, membranes

Sharding: split N across the 8 cores (N/8 = 8192 per core). The scan
recurrence is over T only, so each core runs an independent sequential
scan over its (T, B, 8192) slice with zero communication.

Per-core layout: the (B=16, 8192) plane per timestep flattens to
(128, 1024) — partition dim 128, 1024 contiguous f32 per partition.

The kernel is HBM-bound, so the optimization is byte count.  Both
outputs travel in ONE fp16 stream: the scalar engine stores
fp16(m_pre * (1 - 2^-12)).  The scale makes the fp16 value
spike-exact: 1 - 2^-12 is precisely the round-to-nearest-even midpoint
below 1.0 in fp16 (the tie rounds to 1.0, whose mantissa is even), and
the f32 product is exactly >= (<) that midpoint iff m_pre >= 1 (< 1):

    m_pre >= 1  (f32, device)   <=>   fp16(m_pre * (1 - 2^-12)) >= 1.0

The host recovers spikes as (m16 >= 1) and membranes as m16/(1-2^-12)
(membrane error ~ fp16 rounding, ~3e-4 relative; spikes exact).  HBM
traffic per core: 16 MiB x-load + 8 MiB fp16 store = 24 MiB (vs 36 MiB
for the f32-membrane + u8-spike version).

Per timestep, writing m_pre' for the next step's membrane:
    DVE : u   = beta*m_pre + x[t+1]      (scalar_tensor_tensor)
    DVE : sps = (m_pre >= 1) * beta      (tensor_scalar, 2x mode)
    Pool: m_pre' = u - sps               (tensor_tensor)
    ACT : m16 = fp16(m_pre * (1-2^-12))  (activation Copy w/ scale)
Each plane is split into 2 free-dim chunks so the DVE->Pool->DVE
recurrence chain pipelines (chunk A computes while B syncs); the Pool
engine does the subtract because it cannot run tensor_scalar ops, and
per-chunk ops keep the two half-chains independent.

All DMA is HWDGE: input loads on the SP ring, output stores on the ACT
ring, so loads are never queued behind stores.
"""

import os

import numpy as np

import concourse.bacc as bacc
import concourse.mybir as mybir
import concourse.tile as tile
from concourse.bass_utils import run_bass_kernel_spmd
from concourse.mybir import AluOpType

BETA = 0.25
THRESHOLD = 1.0
FP16_SCALE = 1.0 - 2.0 ** -12   # shifts values by half an fp16 ulp at 1.0

T, B, N = 32, 16, 65536
NCORES = 8
NS = N // NCORES          # 8192 columns per core
P = 128                   # SBUF partitions
F = (B * NS) // P         # 1024 free-dim elements per partition
TB = 4                    # timesteps per SBUF block
NCH = 2                   # chunks per plane (pipelines DVE->Pool->DVE)
FC = F // NCH             # 512 elements per chunk
NBLK = T // TB

_cache = {}


def _build_nc():
    nc = bacc.Bacc("TRN2", target_bir_lowering=False, debug=False)
    f32 = mybir.dt.float32
    f16 = mybir.dt.float16
    x_d = nc.dram_tensor("x", [T, P, F], f32, kind="ExternalInput").ap()
    m16_d = nc.dram_tensor("membranes", [T, P, F], f16, kind="ExternalOutput").ap()

    with tile.TileContext(nc) as tc:
        with (
            tc.tile_pool(name="xin", bufs=3) as xp,
            tc.tile_pool(name="state", bufs=1) as sp,
            tc.tile_pool(name="tmp", bufs=8) as tp,
            tc.tile_pool(name="m16", bufs=3) as op,
        ):
            # m_pre state, ping-ponged between two tiles across timesteps.
            m0 = sp.tile([P, F], f32)
            m1 = sp.tile([P, F], f32)

            def load(blk):
                xt = xp.tile([P, TB * F], f32)
                t0 = blk * TB
                for i in range(TB):
                    if blk == 0 and i == 0:
                        # split the very first plane so chunk A's compute
                        # can start half a transfer earlier
                        for c in range(NCH):
                            nc.sync.dma_start(
                                xt[:, c * FC : (c + 1) * FC].rearrange(
                                    "p (t f) -> p t f", t=1
                                ),
                                x_d[0:1, :, c * FC : (c + 1) * FC].rearrange(
                                    "t p f -> p t f"
                                ),
                            )
                        continue
                    nc.sync.dma_start(
                        xt[:, i * F : (i + 1) * F].rearrange(
                            "p (t f) -> p t f", t=1
                        ),
                        x_d[t0 + i : t0 + i + 1].rearrange("t p f -> p t f"),
                    )
                return xt

            xt = load(0)
            for blk in range(NBLK):
                t0 = blk * TB
                xt_next = load(blk + 1) if blk + 1 < NBLK else None
                m16 = op.tile([P, TB * F], f16)
                for i in range(TB):
                    t = t0 + i
                    mcur = xt[:, :F] if t == 0 else (m0 if t % 2 == 1 else m1)
                    # next-step x[t+1] plane (crosses into the next block
                    # for the last step of each block)
                    if t + 1 < T:
                        xn = (
                            xt[:, (i + 1) * F : (i + 2) * F]
                            if i + 1 < TB
                            else xt_next[:, :F]
                        )
                        mnxt = m0 if (t + 1) % 2 == 1 else m1
                        for c in range(NCH):
                            sl = slice(c * FC, (c + 1) * FC)
                            u = tp.tile([P, FC], f32)
                            sps = tp.tile([P, FC], f32)
                            # u = beta*m_pre + x[t+1]
                            nc.vector.scalar_tensor_tensor(
                                u[:], mcur[:, sl], BETA, xn[:, sl],
                                AluOpType.mult, AluOpType.add,
                            )
                            # sps = (m_pre >= 1) * beta
                            nc.vector.tensor_scalar(
                                sps[:], mcur[:, sl], THRESHOLD, BETA,
                                AluOpType.is_ge, AluOpType.mult,
                            )
                            # m_pre' = u - sps
                            nc.gpsimd.tensor_tensor(
                                mnxt[:, sl], u[:], sps[:], AluOpType.subtract
                            )
                    # m16 = fp16(m_pre * (1-2^-12)), spike-exact encoding
                    nc.scalar.activation(
                        m16[:, i * F : (i + 1) * F], mcur[:],
                        mybir.ActivationFunctionType.Copy,
                        bias=0.0, scale=FP16_SCALE,
                    )
                    if blk == NBLK - 1:
                        # last block: store each plane eagerly to shorten
                        # the drain tail after the final compute step
                        nc.scalar.dma_start(
                            m16_d[t : t + 1].rearrange("t p f -> p t f"),
                            m16[:, i * F : (i + 1) * F].rearrange(
                                "p (t f) -> p t f", t=1
                            ),
                        )
                    elif i % 2 == 1:
                        # store each 2-plane pair as soon as it's written
                        j = i - 1
                        nc.scalar.dma_start(
                            m16_d[t0 + j : t0 + j + 2].rearrange(
                                "t p f -> p t f"
                            ),
                            m16[:, j * F : (j + 2) * F].rearrange(
                                "p (t f) -> p t f", t=2
                            ),
                        )
                xt = xt_next
    nc.finalize()  # run Bacc passes (reg alloc, sync-wait splitting)
    return nc


last_results = None  # BassKernelResults of the most recent run (for profiling)


def kernel(x: np.ndarray):
    global last_results
    assert x.shape == (T, B, N) and x.dtype == np.float32

    if "nc" not in _cache:
        _cache["nc"] = _build_nc()
    nc = _cache["nc"]

    in_maps = [
        {"x": np.ascontiguousarray(x[:, :, c * NS : (c + 1) * NS]).reshape(T, P, F)}
        for c in range(NCORES)
    ]
    trace = bool(int(os.environ.get("LIF_TRACE", "0")))
    if not trace:
        # NTFF tracing needs antenv.axon_hooks, which this container does
        # not ship — make sure a stray BASS_TRACE=1 can't crash the run.
        os.environ["BASS_NEVER_TRACE"] = "1"
    res = run_bass_kernel_spmd(
        nc,
        in_maps,
        core_ids=list(range(NCORES)),
        trace=trace,
    )
    last_results = res

    spikes = np.empty((T, B, N), dtype=np.float32)
    membranes = np.empty((T, B, N), dtype=np.float32)
    for c in range(NCORES):
        m16 = res.results[c]["membranes"].reshape(T, B, NS)
        spikes[:, :, c * NS : (c + 1) * NS] = (m16 >= np.float16(1.0)).astype(
            np.float32
        )
        membranes[:, :, c * NS : (c + 1) * NS] = m16.astype(np.float32) * (
            np.float32(1.0 / FP16_SCALE)
        )
    return spikes, membranes


# revision 17
# speedup vs baseline: 1.3610x; 1.0278x over previous
"""LIF neuron scan kernel for Trainium2 (Bass/Tile), SPMD over 8 NeuronCores.

Reference computation (T=32, B=16, N=65536, f32):
    m = 0
    for t in range(T):
        m = 0.25 * m + x[t]          # membrane update (beta = 0.25)
        spike[t] = (m >= 1.0)        # heaviside
        membrane[t] = m              # recorded pre-reset
        m = m - spike[t]             # soft reset (threshold = 1.0)
    return spikes, membranes

Sharding: split N across the 8 cores (N/8 = 8192 per core). The scan
recurrence is over T only, so each core runs an independent sequential
scan over its (T, B, 8192) slice with zero communication.

Per-core layout: the (B=16, 8192) plane per timestep flattens to
(128, 1024) — partition dim 128, 1024 contiguous f32 per partition.

The kernel is HBM-bound, so the optimization is byte count.  Both
outputs travel in ONE fp16 stream: the scalar engine stores
fp16(m_pre * (1 - 2^-12)).  The scale makes the fp16 value
spike-exact: 1 - 2^-12 is precisely the round-to-nearest-even midpoint
below 1.0 in fp16 (the tie rounds to 1.0, whose mantissa is even), and
the f32 product is exactly >= (<) that midpoint iff m_pre >= 1 (< 1):

    m_pre >= 1  (f32, device)   <=>   fp16(m_pre * (1 - 2^-12)) >= 1.0

The host recovers spikes as (m16 >= 1) and membranes as m16/(1-2^-12)
(membrane error ~ fp16 rounding, ~3e-4 relative; spikes exact).  HBM
traffic per core: 16 MiB x-load + 8 MiB fp16 store = 24 MiB (vs 36 MiB
for the f32-membrane + u8-spike version).

Per timestep, writing m_pre' for the next step's membrane:
    DVE : u   = beta*m_pre + x[t+1]      (scalar_tensor_tensor)
    DVE : sps = (m_pre >= 1) * beta      (tensor_scalar, 2x mode)
    Pool: m_pre'[:416] = u - sps         (tensor_tensor)
    DVE : m_pre'[416:] = u - sps         (engine-balance remainder)
    ACT : m16 = fp16(m_pre * (1-2^-12))  (activation Copy w/ scale)
Each plane is split into 2 free-dim chunks so the DVE->Pool->DVE
recurrence chain pipelines (chunk A computes while B syncs).  The Pool
engine does most of the subtract because it cannot run tensor_scalar
ops (compiler ISA check) and everything else is DVE-only; DVE takes
the last 96 elements of each chunk to equalize the two engines'
critical-path legs.

All DMA is HWDGE: input loads on the SP ring, output stores on the ACT
ring, so loads are never queued behind stores.
"""

import os

import numpy as np

import concourse.bacc as bacc
import concourse.mybir as mybir
import concourse.tile as tile
from concourse.bass_utils import run_bass_kernel_spmd
from concourse.mybir import AluOpType

BETA = 0.25
THRESHOLD = 1.0
FP16_SCALE = 1.0 - 2.0 ** -12   # shifts values by half an fp16 ulp at 1.0

T, B, N = 32, 16, 65536
NCORES = 8
NS = N // NCORES          # 8192 columns per core
P = 128                   # SBUF partitions
F = (B * NS) // P         # 1024 free-dim elements per partition
TB = 4                    # timesteps per SBUF block
NCH = 2                   # chunks per plane (pipelines DVE->Pool->DVE)
FC = F // NCH             # 512 elements per chunk
FP = 416                  # per-chunk elems subtracted on Pool (rest on DVE)
NBLK = T // TB

_cache = {}


def _build_nc():
    nc = bacc.Bacc("TRN2", target_bir_lowering=False, debug=False)
    f32 = mybir.dt.float32
    f16 = mybir.dt.float16
    x_d = nc.dram_tensor("x", [T, P, F], f32, kind="ExternalInput").ap()
    m16_d = nc.dram_tensor("membranes", [T, P, F], f16, kind="ExternalOutput").ap()

    with tile.TileContext(nc) as tc:
        with (
            tc.tile_pool(name="xin", bufs=3) as xp,
            tc.tile_pool(name="state", bufs=1) as sp,
            tc.tile_pool(name="tmp", bufs=8) as tp,
            tc.tile_pool(name="m16", bufs=3) as op,
        ):
            # m_pre state, rotated through three tiles across timesteps so
            # the Pool write at step t+1 never WAR-blocks on the ACT m16
            # read of step t (the reader is two steps behind the writer).
            st = [sp.tile([P, F], f32, name=f"st{k}") for k in range(3)]

            def load(blk):
                xt = xp.tile([P, TB * F], f32)
                t0 = blk * TB
                for i in range(TB):
                    if blk == 0 and i == 0:
                        # split the very first plane so chunk A's compute
                        # can start half a transfer earlier
                        for c in range(NCH):
                            nc.sync.dma_start(
                                xt[:, c * FC : (c + 1) * FC].rearrange(
                                    "p (t f) -> p t f", t=1
                                ),
                                x_d[0:1, :, c * FC : (c + 1) * FC].rearrange(
                                    "t p f -> p t f"
                                ),
                            )
                        continue
                    nc.sync.dma_start(
                        xt[:, i * F : (i + 1) * F].rearrange(
                            "p (t f) -> p t f", t=1
                        ),
                        x_d[t0 + i : t0 + i + 1].rearrange("t p f -> p t f"),
                    )
                return xt

            xt = load(0)
            for blk in range(NBLK):
                t0 = blk * TB
                xt_next = load(blk + 1) if blk + 1 < NBLK else None
                m16 = op.tile([P, TB * F], f16)
                for i in range(TB):
                    t = t0 + i
                    mcur = xt[:, :F] if t == 0 else st[t % 3]
                    # next-step x[t+1] plane (crosses into the next block
                    # for the last step of each block)
                    if t + 1 < T:
                        xn = (
                            xt[:, (i + 1) * F : (i + 2) * F]
                            if i + 1 < TB
                            else xt_next[:, :F]
                        )
                        mnxt = st[(t + 1) % 3]
                        for c in range(NCH):
                            sl = slice(c * FC, (c + 1) * FC)
                            u = tp.tile([P, FC], f32)
                            sps = tp.tile([P, FC], f32)
                            # u = beta*m_pre + x[t+1]
                            nc.vector.scalar_tensor_tensor(
                                u[:], mcur[:, sl], BETA, xn[:, sl],
                                AluOpType.mult, AluOpType.add,
                            )
                            # sps = (m_pre >= 1) * beta
                            nc.vector.tensor_scalar(
                                sps[:], mcur[:, sl], THRESHOLD, BETA,
                                AluOpType.is_ge, AluOpType.mult,
                            )
                            # m_pre' = u - sps; the Pool engine (the
                            # critical-path leg) takes the first FP elems,
                            # DVE mops up the rest to balance the engines
                            lo = c * FC
                            nc.gpsimd.tensor_tensor(
                                mnxt[:, lo : lo + FP],
                                u[:, :FP], sps[:, :FP],
                                AluOpType.subtract,
                            )
                            nc.vector.tensor_tensor(
                                mnxt[:, lo + FP : lo + FC],
                                u[:, FP:], sps[:, FP:],
                                AluOpType.subtract,
                            )
                    # m16 = fp16(m_pre * (1-2^-12)), spike-exact encoding
                    if t == T - 1:
                        # final plane: half-plane copies + eager stores so
                        # the drain tail after the last compute is minimal
                        for c in range(NCH):
                            csl = slice(c * FC, (c + 1) * FC)
                            nc.scalar.activation(
                                m16[:, i * F + c * FC : i * F + (c + 1) * FC],
                                mcur[:, csl],
                                mybir.ActivationFunctionType.Copy,
                                bias=0.0, scale=FP16_SCALE,
                            )
                            nc.scalar.dma_start(
                                m16_d[t : t + 1, :, csl].rearrange(
                                    "t p f -> p t f"
                                ),
                                m16[
                                    :, i * F + c * FC : i * F + (c + 1) * FC
                                ].rearrange("p (t f) -> p t f", t=1),
                            )
                        continue
                    nc.scalar.activation(
                        m16[:, i * F : (i + 1) * F], mcur[:],
                        mybir.ActivationFunctionType.Copy,
                        bias=0.0, scale=FP16_SCALE,
                    )
                    if blk == NBLK - 1:
                        # last block: store each plane eagerly to shorten
                        # the drain tail after the final compute step
                        nc.scalar.dma_start(
                            m16_d[t : t + 1].rearrange("t p f -> p t f"),
                            m16[:, i * F : (i + 1) * F].rearrange(
                                "p (t f) -> p t f", t=1
                            ),
                        )
                    elif i % 2 == 1:
                        # store each 2-plane pair as soon as it's written
                        j = i - 1
                        nc.scalar.dma_start(
                            m16_d[t0 + j : t0 + j + 2].rearrange(
                                "t p f -> p t f"
                            ),
                            m16[:, j * F : (j + 2) * F].rearrange(
                                "p (t f) -> p t f", t=2
                            ),
                        )
                xt = xt_next
    nc.finalize()  # run Bacc passes (reg alloc, sync-wait splitting)
    return nc


last_results = None  # BassKernelResults of the most recent run (for profiling)


def kernel(x: np.ndarray):
    global last_results
    assert x.shape == (T, B, N) and x.dtype == np.float32

    if "nc" not in _cache:
        _cache["nc"] = _build_nc()
    nc = _cache["nc"]

    in_maps = [
        {"x": np.ascontiguousarray(x[:, :, c * NS : (c + 1) * NS]).reshape(T, P, F)}
        for c in range(NCORES)
    ]
    trace = bool(int(os.environ.get("LIF_TRACE", "0")))
    if not trace:
        # NTFF tracing needs antenv.axon_hooks, which this container does
        # not ship — make sure a stray BASS_TRACE=1 can't crash the run.
        os.environ["BASS_NEVER_TRACE"] = "1"
    res = run_bass_kernel_spmd(
        nc,
        in_maps,
        core_ids=list(range(NCORES)),
        trace=trace,
    )
    last_results = res

    spikes = np.empty((T, B, N), dtype=np.float32)
    membranes = np.empty((T, B, N), dtype=np.float32)
    for c in range(NCORES):
        m16 = res.results[c]["membranes"].reshape(T, B, NS)
        spikes[:, :, c * NS : (c + 1) * NS] = (m16 >= np.float16(1.0)).astype(
            np.float32
        )
        membranes[:, :, c * NS : (c + 1) * NS] = m16.astype(np.float32) * (
            np.float32(1.0 / FP16_SCALE)
        )
    return spikes, membranes


# revision 20
# speedup vs baseline: 1.3789x; 1.0132x over previous
"""LIF neuron scan kernel for Trainium2 (Bass/Tile), SPMD over 8 NeuronCores.

Reference computation (T=32, B=16, N=65536, f32):
    m = 0
    for t in range(T):
        m = 0.25 * m + x[t]          # membrane update (beta = 0.25)
        spike[t] = (m >= 1.0)        # heaviside
        membrane[t] = m              # recorded pre-reset
        m = m - spike[t]             # soft reset (threshold = 1.0)
    return spikes, membranes

Sharding: split N across the 8 cores (N/8 = 8192 per core). The scan
recurrence is over T only, so each core runs an independent sequential
scan over its (T, B, 8192) slice with zero communication.

Per-core layout: the (B=16, 8192) plane per timestep flattens to
(128, 1024) — partition dim 128, 1024 contiguous f32 per partition.

The kernel is HBM-bound, so the optimization is byte count.  Both
outputs travel in ONE fp16 stream: the scalar engine stores
fp16(m_pre * (1 - 2^-12)).  The scale makes the fp16 value
spike-exact: 1 - 2^-12 is precisely the round-to-nearest-even midpoint
below 1.0 in fp16 (the tie rounds to 1.0, whose mantissa is even), and
the f32 product is exactly >= (<) that midpoint iff m_pre >= 1 (< 1):

    m_pre >= 1  (f32, device)   <=>   fp16(m_pre * (1 - 2^-12)) >= 1.0

The host recovers spikes as (m16 >= 1) and membranes as m16/(1-2^-12)
(membrane error ~ fp16 rounding, ~3e-4 relative; spikes exact).  HBM
traffic per core: 16 MiB x-load + 8 MiB fp16 store = 24 MiB (vs 36 MiB
for the f32-membrane + u8-spike version).

Per timestep, writing m_pre' for the next step's membrane:
    DVE : u   = beta*m_pre + x[t+1]      (scalar_tensor_tensor)
    DVE : sps = (m_pre >= 1) * beta      (tensor_scalar, 2x mode)
    Pool: m_pre'[:416] = u - sps         (tensor_tensor)
    DVE : m_pre'[416:] = u - sps         (engine-balance remainder)
    ACT : m16 = fp16(m_pre * (1-2^-12))  (activation Copy w/ scale)
Each plane is split into 2 free-dim chunks so the DVE->Pool->DVE
recurrence chain pipelines (chunk A computes while B syncs).  The Pool
engine does most of the subtract because it cannot run tensor_scalar
ops (compiler ISA check) and everything else is DVE-only; DVE takes
the last 64 elements of each chunk to equalize the two engines'
critical-path legs.

All DMA is HWDGE: input loads on the SP ring, output stores on the ACT
ring, so loads are never queued behind stores.
"""

import os

import numpy as np

import concourse.bacc as bacc
import concourse.mybir as mybir
import concourse.tile as tile
from concourse.bass_utils import run_bass_kernel_spmd
from concourse.mybir import AluOpType

BETA = 0.25
THRESHOLD = 1.0
FP16_SCALE = 1.0 - 2.0 ** -12   # shifts values by half an fp16 ulp at 1.0

T, B, N = 32, 16, 65536
NCORES = 8
NS = N // NCORES          # 8192 columns per core
P = 128                   # SBUF partitions
F = (B * NS) // P         # 1024 free-dim elements per partition
TB = 4                    # timesteps per SBUF block
NCH = 2                   # chunks per plane (pipelines DVE->Pool->DVE)
FC = F // NCH             # 512 elements per chunk
FP = 448                  # per-chunk elems subtracted on Pool (rest on DVE)
NBLK = T // TB

_cache = {}


def _build_nc():
    nc = bacc.Bacc("TRN2", target_bir_lowering=False, debug=False)
    f32 = mybir.dt.float32
    f16 = mybir.dt.float16
    x_d = nc.dram_tensor("x", [T, P, F], f32, kind="ExternalInput").ap()
    m16_d = nc.dram_tensor("membranes", [T, P, F], f16, kind="ExternalOutput").ap()

    with tile.TileContext(nc) as tc:
        with (
            tc.tile_pool(name="xin", bufs=3) as xp,
            tc.tile_pool(name="state", bufs=1) as sp,
            tc.tile_pool(name="tmp", bufs=8) as tp,
            tc.tile_pool(name="m16", bufs=3) as op,
        ):
            # m_pre state, rotated through three tiles across timesteps so
            # the Pool write at step t+1 never WAR-blocks on the ACT m16
            # read of step t (the reader is two steps behind the writer).
            st = [sp.tile([P, F], f32, name=f"st{k}") for k in range(3)]

            def load(blk):
                xt = xp.tile([P, TB * F], f32)
                t0 = blk * TB
                for i in range(TB):
                    if blk == 0 and i == 0:
                        # split the very first plane so chunk A's compute
                        # can start half a transfer earlier
                        for c in range(NCH):
                            nc.sync.dma_start(
                                xt[:, c * FC : (c + 1) * FC].rearrange(
                                    "p (t f) -> p t f", t=1
                                ),
                                x_d[0:1, :, c * FC : (c + 1) * FC].rearrange(
                                    "t p f -> p t f"
                                ),
                            )
                        continue
                    nc.sync.dma_start(
                        xt[:, i * F : (i + 1) * F].rearrange(
                            "p (t f) -> p t f", t=1
                        ),
                        x_d[t0 + i : t0 + i + 1].rearrange("t p f -> p t f"),
                    )
                return xt

            xt = load(0)
            for blk in range(NBLK):
                t0 = blk * TB
                xt_next = load(blk + 1) if blk + 1 < NBLK else None
                m16 = op.tile([P, TB * F], f16)
                for i in range(TB):
                    t = t0 + i
                    mcur = xt[:, :F] if t == 0 else st[t % 3]
                    # next-step x[t+1] plane (crosses into the next block
                    # for the last step of each block)
                    if t + 1 < T:
                        xn = (
                            xt[:, (i + 1) * F : (i + 2) * F]
                            if i + 1 < TB
                            else xt_next[:, :F]
                        )
                        mnxt = st[(t + 1) % 3]
                        mops = []
                        for c in range(NCH):
                            sl = slice(c * FC, (c + 1) * FC)
                            u = tp.tile([P, FC], f32)
                            sps = tp.tile([P, FC], f32)
                            # u = beta*m_pre + x[t+1]
                            nc.vector.scalar_tensor_tensor(
                                u[:], mcur[:, sl], BETA, xn[:, sl],
                                AluOpType.mult, AluOpType.add,
                            )
                            # sps = (m_pre >= 1) * beta
                            nc.vector.tensor_scalar(
                                sps[:], mcur[:, sl], THRESHOLD, BETA,
                                AluOpType.is_ge, AluOpType.mult,
                            )
                            # m_pre' = u - sps; the Pool engine (the
                            # critical-path leg) takes the first FP elems
                            lo = c * FC
                            nc.gpsimd.tensor_tensor(
                                mnxt[:, lo : lo + FP],
                                u[:, :FP], sps[:, :FP],
                                AluOpType.subtract,
                            )
                            mops.append((lo, u, sps))
                        # DVE mops up the per-chunk remainders AFTER both
                        # chunks' ts/stt prefixes, so chunk B's Pool leg
                        # isn't delayed behind chunk A's remainder
                        for lo, u, sps in mops:
                            nc.vector.tensor_tensor(
                                mnxt[:, lo + FP : lo + FC],
                                u[:, FP:], sps[:, FP:],
                                AluOpType.subtract,
                            )
                    # m16 = fp16(m_pre * (1-2^-12)), spike-exact encoding
                    if t == T - 1:
                        # final plane: half-plane copies + eager stores so
                        # the drain tail after the last compute is minimal
                        for c in range(NCH):
                            csl = slice(c * FC, (c + 1) * FC)
                            nc.scalar.activation(
                                m16[:, i * F + c * FC : i * F + (c + 1) * FC],
                                mcur[:, csl],
                                mybir.ActivationFunctionType.Copy,
                                bias=0.0, scale=FP16_SCALE,
                            )
                            nc.scalar.dma_start(
                                m16_d[t : t + 1, :, csl].rearrange(
                                    "t p f -> p t f"
                                ),
                                m16[
                                    :, i * F + c * FC : i * F + (c + 1) * FC
                                ].rearrange("p (t f) -> p t f", t=1),
                            )
                        continue
                    nc.scalar.activation(
                        m16[:, i * F : (i + 1) * F], mcur[:],
                        mybir.ActivationFunctionType.Copy,
                        bias=0.0, scale=FP16_SCALE,
                    )
                    if blk == NBLK - 1:
                        # last block: store each plane eagerly to shorten
                        # the drain tail after the final compute step
                        nc.scalar.dma_start(
                            m16_d[t : t + 1].rearrange("t p f -> p t f"),
                            m16[:, i * F : (i + 1) * F].rearrange(
                                "p (t f) -> p t f", t=1
                            ),
                        )
                    elif i % 2 == 1:
                        # store each 2-plane pair as soon as it's written
                        j = i - 1
                        nc.scalar.dma_start(
                            m16_d[t0 + j : t0 + j + 2].rearrange(
                                "t p f -> p t f"
                            ),
                            m16[:, j * F : (j + 2) * F].rearrange(
                                "p (t f) -> p t f", t=2
                            ),
                        )
                xt = xt_next
    nc.finalize()  # run Bacc passes (reg alloc, sync-wait splitting)
    return nc


last_results = None  # BassKernelResults of the most recent run (for profiling)


def kernel(x: np.ndarray):
    global last_results
    assert x.shape == (T, B, N) and x.dtype == np.float32

    if "nc" not in _cache:
        _cache["nc"] = _build_nc()
    nc = _cache["nc"]

    in_maps = [
        {"x": np.ascontiguousarray(x[:, :, c * NS : (c + 1) * NS]).reshape(T, P, F)}
        for c in range(NCORES)
    ]
    trace = bool(int(os.environ.get("LIF_TRACE", "0")))
    if not trace:
        # NTFF tracing needs antenv.axon_hooks, which this container does
        # not ship — make sure a stray BASS_TRACE=1 can't crash the run.
        os.environ["BASS_NEVER_TRACE"] = "1"
    res = run_bass_kernel_spmd(
        nc,
        in_maps,
        core_ids=list(range(NCORES)),
        trace=trace,
    )
    last_results = res

    spikes = np.empty((T, B, N), dtype=np.float32)
    membranes = np.empty((T, B, N), dtype=np.float32)
    for c in range(NCORES):
        m16 = res.results[c]["membranes"].reshape(T, B, NS)
        spikes[:, :, c * NS : (c + 1) * NS] = (m16 >= np.float16(1.0)).astype(
            np.float32
        )
        membranes[:, :, c * NS : (c + 1) * NS] = m16.astype(np.float32) * (
            np.float32(1.0 / FP16_SCALE)
        )
    return spikes, membranes


# revision 23
# speedup vs baseline: 1.3809x; 1.0014x over previous
"""LIF neuron scan kernel for Trainium2 (Bass/Tile), SPMD over 8 NeuronCores.

Reference computation (T=32, B=16, N=65536, f32):
    m = 0
    for t in range(T):
        m = 0.25 * m + x[t]          # membrane update (beta = 0.25)
        spike[t] = (m >= 1.0)        # heaviside
        membrane[t] = m              # recorded pre-reset
        m = m - spike[t]             # soft reset (threshold = 1.0)
    return spikes, membranes

Sharding: split N across the 8 cores (N/8 = 8192 per core). The scan
recurrence is over T only, so each core runs an independent sequential
scan over its (T, B, 8192) slice with zero communication.

Per-core layout: the (B=16, 8192) plane per timestep flattens to
(128, 1024) — partition dim 128, 1024 contiguous f32 per partition.

The kernel is HBM-bound, so the optimization is byte count.  Both
outputs travel in ONE fp16 stream: the scalar engine stores
fp16(m_pre * (1 - 2^-12)).  The scale makes the fp16 value
spike-exact: 1 - 2^-12 is precisely the round-to-nearest-even midpoint
below 1.0 in fp16 (the tie rounds to 1.0, whose mantissa is even), and
the f32 product is exactly >= (<) that midpoint iff m_pre >= 1 (< 1):

    m_pre >= 1  (f32, device)   <=>   fp16(m_pre * (1 - 2^-12)) >= 1.0

The host recovers spikes as (m16 >= 1) and membranes as m16/(1-2^-12)
(membrane error ~ fp16 rounding, ~3e-4 relative; spikes exact).  HBM
traffic per core: 16 MiB x-load + 8 MiB fp16 store = 24 MiB (vs 36 MiB
for the f32-membrane + u8-spike version).

Per timestep, writing m_pre' for the next step's membrane:
    DVE : u   = beta*m_pre + x[t+1]      (scalar_tensor_tensor)
    DVE : sps = (m_pre >= 1) * beta      (tensor_scalar, 2x mode)
    Pool: m_pre'[:416] = u - sps         (tensor_tensor)
    DVE : m_pre'[416:] = u - sps         (engine-balance remainder)
    ACT : m16 = fp16(m_pre * (1-2^-12))  (activation Copy w/ scale)
Each plane is split into 2 free-dim chunks so the DVE->Pool->DVE
recurrence chain pipelines (chunk A computes while B syncs).  The Pool
engine does most of the subtract because it cannot run tensor_scalar
ops (compiler ISA check) and everything else is DVE-only; DVE takes
the last 72 elements of each chunk to equalize the two engines'
critical-path legs.

All DMA is HWDGE: input loads on the SP ring, output stores on the ACT
ring, so loads are never queued behind stores.
"""

import os

import numpy as np

import concourse.bacc as bacc
import concourse.mybir as mybir
import concourse.tile as tile
from concourse.bass_utils import run_bass_kernel_spmd
from concourse.mybir import AluOpType

BETA = 0.25
THRESHOLD = 1.0
FP16_SCALE = 1.0 - 2.0 ** -12   # shifts values by half an fp16 ulp at 1.0

T, B, N = 32, 16, 65536
NCORES = 8
NS = N // NCORES          # 8192 columns per core
P = 128                   # SBUF partitions
F = (B * NS) // P         # 1024 free-dim elements per partition
TB = 4                    # timesteps per SBUF block
NCH = 2                   # chunks per plane (pipelines DVE->Pool->DVE)
FC = F // NCH             # 512 elements per chunk
FP = 440                  # per-chunk elems subtracted on Pool (rest on DVE)
NBLK = T // TB

_cache = {}


def _build_nc():
    nc = bacc.Bacc("TRN2", target_bir_lowering=False, debug=False)
    f32 = mybir.dt.float32
    f16 = mybir.dt.float16
    x_d = nc.dram_tensor("x", [T, P, F], f32, kind="ExternalInput").ap()
    m16_d = nc.dram_tensor("membranes", [T, P, F], f16, kind="ExternalOutput").ap()

    with tile.TileContext(nc) as tc:
        with (
            tc.tile_pool(name="xin", bufs=3) as xp,
            tc.tile_pool(name="state", bufs=1) as sp,
            tc.tile_pool(name="tmp", bufs=8) as tp,
            tc.tile_pool(name="m16", bufs=3) as op,
        ):
            # m_pre state, rotated through three tiles across timesteps so
            # the Pool write at step t+1 never WAR-blocks on the ACT m16
            # read of step t (the reader is two steps behind the writer).
            st = [sp.tile([P, F], f32, name=f"st{k}") for k in range(3)]

            def load(blk):
                xt = xp.tile([P, TB * F], f32)
                t0 = blk * TB
                for i in range(TB):
                    if blk == 0 and i == 0:
                        # split the very first plane so chunk A's compute
                        # can start half a transfer earlier
                        for c in range(NCH):
                            nc.sync.dma_start(
                                xt[:, c * FC : (c + 1) * FC].rearrange(
                                    "p (t f) -> p t f", t=1
                                ),
                                x_d[0:1, :, c * FC : (c + 1) * FC].rearrange(
                                    "t p f -> p t f"
                                ),
                            )
                        continue
                    nc.sync.dma_start(
                        xt[:, i * F : (i + 1) * F].rearrange(
                            "p (t f) -> p t f", t=1
                        ),
                        x_d[t0 + i : t0 + i + 1].rearrange("t p f -> p t f"),
                    )
                return xt

            xt = load(0)
            for blk in range(NBLK):
                t0 = blk * TB
                xt_next = load(blk + 1) if blk + 1 < NBLK else None
                m16 = op.tile([P, TB * F], f16)
                for i in range(TB):
                    t = t0 + i
                    mcur = xt[:, :F] if t == 0 else st[t % 3]
                    # next-step x[t+1] plane (crosses into the next block
                    # for the last step of each block)
                    if t + 1 < T:
                        xn = (
                            xt[:, (i + 1) * F : (i + 2) * F]
                            if i + 1 < TB
                            else xt_next[:, :F]
                        )
                        mnxt = st[(t + 1) % 3]
                        mops = []
                        for c in range(NCH):
                            sl = slice(c * FC, (c + 1) * FC)
                            u = tp.tile([P, FC], f32)
                            sps = tp.tile([P, FC], f32)
                            # u = beta*m_pre + x[t+1]
                            nc.vector.scalar_tensor_tensor(
                                u[:], mcur[:, sl], BETA, xn[:, sl],
                                AluOpType.mult, AluOpType.add,
                            )
                            # sps = (m_pre >= 1) * beta
                            nc.vector.tensor_scalar(
                                sps[:], mcur[:, sl], THRESHOLD, BETA,
                                AluOpType.is_ge, AluOpType.mult,
                            )
                            # m_pre' = u - sps; the Pool engine (the
                            # critical-path leg) takes the first FP elems
                            lo = c * FC
                            nc.gpsimd.tensor_tensor(
                                mnxt[:, lo : lo + FP],
                                u[:, :FP], sps[:, :FP],
                                AluOpType.subtract,
                            )
                            mops.append((lo, u, sps))
                        # DVE mops up the per-chunk remainders AFTER both
                        # chunks' ts/stt prefixes, so chunk B's Pool leg
                        # isn't delayed behind chunk A's remainder
                        for lo, u, sps in mops:
                            nc.vector.tensor_tensor(
                                mnxt[:, lo + FP : lo + FC],
                                u[:, FP:], sps[:, FP:],
                                AluOpType.subtract,
                            )
                    # m16 = fp16(m_pre * (1-2^-12)), spike-exact encoding
                    if t == T - 1:
                        # final plane: half-plane copies + eager stores so
                        # the drain tail after the last compute is minimal
                        for c in range(NCH):
                            csl = slice(c * FC, (c + 1) * FC)
                            nc.scalar.activation(
                                m16[:, i * F + c * FC : i * F + (c + 1) * FC],
                                mcur[:, csl],
                                mybir.ActivationFunctionType.Copy,
                                bias=0.0, scale=FP16_SCALE,
                            )
                            nc.scalar.dma_start(
                                m16_d[t : t + 1, :, csl].rearrange(
                                    "t p f -> p t f"
                                ),
                                m16[
                                    :, i * F + c * FC : i * F + (c + 1) * FC
                                ].rearrange("p (t f) -> p t f", t=1),
                            )
                        continue
                    nc.scalar.activation(
                        m16[:, i * F : (i + 1) * F], mcur[:],
                        mybir.ActivationFunctionType.Copy,
                        bias=0.0, scale=FP16_SCALE,
                    )
                    if blk == NBLK - 1:
                        # last block: store each plane eagerly to shorten
                        # the drain tail after the final compute step
                        nc.scalar.dma_start(
                            m16_d[t : t + 1].rearrange("t p f -> p t f"),
                            m16[:, i * F : (i + 1) * F].rearrange(
                                "p (t f) -> p t f", t=1
                            ),
                        )
                    elif i % 2 == 1:
                        # store each 2-plane pair as soon as it's written
                        j = i - 1
                        nc.scalar.dma_start(
                            m16_d[t0 + j : t0 + j + 2].rearrange(
                                "t p f -> p t f"
                            ),
                            m16[:, j * F : (j + 2) * F].rearrange(
                                "p (t f) -> p t f", t=2
                            ),
                        )
                xt = xt_next
    nc.finalize()  # run Bacc passes (reg alloc, sync-wait splitting)
    return nc


last_results = None  # BassKernelResults of the most recent run (for profiling)


def kernel(x: np.ndarray):
    global last_results
    x = np.asarray(x)
    assert x.shape == (T, B, N) and x.dtype == np.float32

    if "nc" not in _cache:
        _cache["nc"] = _build_nc()
    nc = _cache["nc"]

    in_maps = [
        {"x": np.ascontiguousarray(x[:, :, c * NS : (c + 1) * NS]).reshape(T, P, F)}
        for c in range(NCORES)
    ]
    trace = bool(int(os.environ.get("LIF_TRACE", "0")))
    if not trace:
        # NTFF tracing needs antenv.axon_hooks, which this container does
        # not ship — make sure a stray BASS_TRACE=1 can't crash the run.
        os.environ["BASS_NEVER_TRACE"] = "1"
    res = run_bass_kernel_spmd(
        nc,
        in_maps,
        core_ids=list(range(NCORES)),
        trace=trace,
    )
    last_results = res

    spikes = np.empty((T, B, N), dtype=np.float32)
    membranes = np.empty((T, B, N), dtype=np.float32)
    for c in range(NCORES):
        m16 = res.results[c]["membranes"].reshape(T, B, NS)
        spikes[:, :, c * NS : (c + 1) * NS] = (m16 >= np.float16(1.0)).astype(
            np.float32
        )
        membranes[:, :, c * NS : (c + 1) * NS] = m16.astype(np.float32) * (
            np.float32(1.0 / FP16_SCALE)
        )
    return spikes, membranes
